# revision 20
# baseline (speedup 1.0000x reference)
"""DifferentiableStack kernel for 8 Trainium2 NeuronCores.

Reformulation: unrolling the mem recurrence
    mem_t = push_t * outer(u_t, v_t) + (1 - push_t) * mem_{t-1}
    read_t = mem_t^T ptr_t
gives
    reads = G @ V + c ⊙ (P @ mem0),   G = W ⊙ (P U^T)  (lower triangular)
with
    P[t] = ptr_t            (normalized pointer after step t)
    U[t] = u_t = roll(ptr_{t-1}, 1)
    W[t,k] = push_k * Π_{j=k+1..t} (1 - push_j)   (k ≤ t, else 0)
    c[t]   = Π_{j=1..t} (1 - push_j)

The ptr chain / W / G are small sequential scalar recurrences plus one
[T,S]x[S,T] product, computed on the host in float64.  The heavy
O(T^2 D) contraction reads = G @ V runs on the TensorEngines in exact
fp32, sharded along the d (feature) axis: core i computes
readsT[d_shard, t] = Σ_k V[k, d_shard]^T G^T[k, t].  G^T[k, t] = 0 for
t < k, so the k-chunk matmuls accumulate into shrinking t-ranges
(N = 512, 384, 256, 128) and the zero triangle is never transferred.
"""

import sys

sys.path.insert(0, "/opt/trn_rl_repo")

import numpy as np

T, S, D = 512, 256, 1024
EPS = 1e-6
NCORES = 8
DSH = D // NCORES  # 128 features per core
P_PART = 128  # SBUF partitions
N_KC = T // P_PART  # 4 k-chunks

# packed input layout, one [128, 1792] f32 array per core:
#   | V0 128 | GT0 512 | V1 128 | GT1 384 | V2 128 | GT2 256 | V3 128 | GT3 128 |
# where Vk = V[k-chunk, d_shard] and GTk = G^T[k-chunk, t >= kc*128].
_V_OFF = [0, 640, 1152, 1536]
_G_OFF = [128, 768, 1280, 1664]
_G_W = [512, 384, 256, 128]
PK_W = 1792

_compiled = {}


def _host_precompute(controls, ptr0):
    """float64 ptr chain + blend-weight matrix. O(T*S + T^2) scalar work."""
    ctrl = controls.astype(np.float64)
    ptr = ptr0.astype(np.float64).copy()
    P = np.empty((T, S), dtype=np.float64)
    U = np.empty((T, S), dtype=np.float64)
    for t in range(T):
        push, pop, noop = ctrl[t, 0], ctrl[t, 1], ctrl[t, 2]
        u = np.roll(ptr, 1)
        d = np.roll(ptr, -1)
        q = push * u + pop * d + noop * ptr
        ptr = q / (q.sum() + EPS)
        U[t] = u
        P[t] = ptr
    push_v = ctrl[:, 0]
    alpha = 1.0 - push_v
    W = np.zeros((T, T), dtype=np.float64)
    row = np.zeros(T, dtype=np.float64)
    for t in range(T):
        row *= alpha[t]
        row[t] = push_v[t]
        W[t] = row
    c = np.cumprod(alpha)
    return P, U, W, c


def _build_program():
    from contextlib import ExitStack

    import concourse.bass as bass
    import concourse.mybir as mybir

    f32 = mybir.dt.float32
    nc = bass.Bass("TRN2", num_devices=NCORES)

    pk = nc.declare_dram_parameter("pk", [P_PART, PK_W], f32, isOutput=False)
    out = nc.declare_dram_parameter("readst", [DSH, T], f32, isOutput=True)

    # Raw Bass with manual semaphores: the walrus build here only accepts
    # ONE sync wait per instruction, so every cross-engine dependency is a
    # standalone wait_ge followed by wait-free compute instructions.
    with ExitStack() as ctx:
        t_pk = ctx.enter_context(nc.sbuf_tensor("t_pk", [P_PART, PK_W], f32))
        t_ot = ctx.enter_context(nc.sbuf_tensor("t_ot", [DSH, T], f32))
        ps_r = ctx.enter_context(nc.psum_tensor("ps_r", [DSH, T], f32))

        # Region 1 = V0|GT0 (cols 0:640) — feeds the first matmul.
        # Region 2 = the rest.  Chunks are spread over three issuing
        # queues (gpsimd SWDGE, sync+scalar HWDGE); a semaphore may not
        # mix SWDGE and HWDGE updates, so each region has one sem per
        # DGE class.
        dR1s = ctx.enter_context(nc.semaphore("dR1s"))
        dR1h = ctx.enter_context(nc.semaphore("dR1h"))
        dR2s = ctx.enter_context(nc.semaphore("dR2s"))
        dR2h = ctx.enter_context(nc.semaphore("dR2h"))
        dOut = ctx.enter_context(nc.semaphore("dOut"))
        sPE = ctx.enter_context(nc.semaphore("sPE"))
        sDVE = ctx.enter_context(nc.semaphore("sDVE"))

        vs = [t_pk[:, _V_OFF[kc] : _V_OFF[kc] + DSH] for kc in range(N_KC)]
        gts = [t_pk[:, _G_OFF[kc] : _G_OFF[kc] + _G_W[kc]] for kc in range(N_KC)]

        def pk_chunk(eng, c0, c1, sem):
            eng.dma_start(t_pk[:, c0:c1], pk[:, c0:c1]).then_inc(sem, 16)

        with nc.Block() as block:

            @block.gpsimd
            def _(g):
                pk_chunk(g, 0, 256, dR1s)  # R1
                pk_chunk(g, 1408, 1792, dR2s)  # R2
                g.wait_ge(sDVE, 1)
                g.dma_start(out[:, :], t_ot[:]).then_inc(dOut, 16)
                g.wait_ge(dOut, 16)

            @block.sync
            def _(sy):
                pk_chunk(sy, 256, 448, dR1h)  # R1
                pk_chunk(sy, 640, 1024, dR2h)  # R2

            @block.scalar
            def _(sc):
                pk_chunk(sc, 448, 640, dR1h)  # R1
                pk_chunk(sc, 1024, 1408, dR2h)  # R2

            @block.tensor
            def _(pe):
                # readsT[d, t] = Σ_kc V[kc]^T G^T[kc] — fp32, exact.
                # G^T[kc] is zero for t < kc*128, so matmul kc only
                # touches psum columns [kc*128 : 512].
                pe.wait_ge(dR1s, 16)
                pe.wait_ge(dR1h, 32)
                pe.matmul(
                    ps_r[:],
                    lhsT=vs[0][:],
                    rhs=gts[0][:],
                    start=True,
                    stop=False,
                ).then_inc(sPE)
                pe.wait_ge(dR2s, 16)
                pe.wait_ge(dR2h, 32)
                for kc in range(1, N_KC):
                    pe.matmul(
                        ps_r[:, kc * P_PART : T],
                        lhsT=vs[kc][:],
                        rhs=gts[kc][:],
                        start=False,
                        stop=(kc == N_KC - 1),
                    ).then_inc(sPE)

            @block.vector
            def _(v):
                v.wait_ge(sPE, N_KC)
                v.tensor_copy(t_ot[:], ps_r[:]).then_inc(sDVE)

    return nc


def _get_program():
    if "nc" not in _compiled:
        _compiled["nc"] = _build_program()
    return _compiled["nc"]


def _make_inputs(values, controls, mem0, ptr0):
    P, U, W, c = _host_precompute(controls, ptr0)
    G = (W * (P @ U.T)).astype(np.float32)  # [t, k], lower triangular
    GT = np.ascontiguousarray(G.T)  # [k, t]
    values = np.asarray(values, dtype=np.float32)

    def pack(core):
        vsh = values[:, core * DSH : (core + 1) * DSH]  # [T, DSH]
        parts = []
        for kc in range(N_KC):
            parts.append(vsh[kc * P_PART : (kc + 1) * P_PART, :])
            parts.append(
                GT[kc * P_PART : (kc + 1) * P_PART, kc * P_PART : T]
            )
        return np.ascontiguousarray(np.concatenate(parts, axis=1), dtype=np.float32)

    in_maps = [{"pk": pack(i)} for i in range(NCORES)]
    return in_maps, P, c


def kernel(values, controls, mem0, ptr0):
    from concourse.bass_utils import run_bass_kernel_spmd

    mem0 = np.asarray(mem0, dtype=np.float32)
    in_maps, P, c = _make_inputs(values, controls, mem0, ptr0)
    nc = _get_program()
    res = run_bass_kernel_spmd(nc, in_maps, list(range(NCORES)))
    reads = np.concatenate(
        [np.asarray(res.results[i]["readst"]).T for i in range(NCORES)], axis=1
    )
    if np.any(mem0):
        reads = reads + (c[:, None] * (P @ mem0.astype(np.float64))).astype(np.float32)
    ptrs = P.astype(np.float32)
    return reads, ptrs


# revision 24
# speedup vs baseline: 1.1175x; 1.1175x over previous
"""DifferentiableStack kernel for 8 Trainium2 NeuronCores.

Reformulation: unrolling the mem recurrence
    mem_t = push_t * outer(u_t, v_t) + (1 - push_t) * mem_{t-1}
    read_t = mem_t^T ptr_t
gives
    reads = G @ V + c ⊙ (P @ mem0),   G = W ⊙ (P U^T)  (lower triangular)
with
    P[t] = ptr_t            (normalized pointer after step t)
    U[t] = u_t = roll(ptr_{t-1}, 1)
    W[t,k] = push_k * Π_{j=k+1..t} (1 - push_j)   (k ≤ t, else 0)
    c[t]   = Π_{j=1..t} (1 - push_j)

The ptr chain / W / G are small sequential scalar recurrences plus one
[T,S]x[S,T] product, computed on the host in float64.  The heavy
O(T^2 D) contraction reads = G @ V runs on the TensorEngines in exact
fp32, sharded along the d (feature) axis: core i computes
readsT[d_shard, t] = Σ_k V[k, d_shard]^T G^T[k, t].  G^T is zero for
t < k and numerically zero for t - k >= 128 (exponential decay of the
blend weights), so only the diagonal and first sub-diagonal 128x128
blocks are transferred and multiplied.
"""

import sys

sys.path.insert(0, "/opt/trn_rl_repo")

import numpy as np

T, S, D = 512, 256, 1024
EPS = 1e-6
NCORES = 8
DSH = D // NCORES  # 128 features per core
P_PART = 128  # SBUF partitions
N_KC = T // P_PART  # 4 k-chunks

# G is numerically block-banded: W decays like Π(1-push_j), so entries
# with t - k >= 128 are < 1e-40 — only the diagonal and first
# sub-diagonal 128x128 blocks of G^T survive (verified < 1.6e-57
# residual on the generator's input distribution; any residual is
# added back on the host, see kernel()).
#
# packed input layout, one [128, 1408] f32 array per core:
#   | V0 | GT0 (t 0:256) | V1 | GT1 (t 128:384) | V2 | GT2 (t 256:512) | V3 | GT3 (t 384:512) |
# where Vk = V[k-chunk, d_shard] and GTk = G^T[k-chunk, t-band].
_V_OFF = [0, 384, 768, 1152]
_G_OFF = [128, 512, 896, 1280]
_G_T0 = [0, 128, 256, 384]
_G_W = [256, 256, 256, 128]
PK_W = 1408

_compiled = {}


def _host_precompute(controls, ptr0):
    """float64 ptr chain + blend-weight matrix. O(T*S + T^2) scalar work."""
    ctrl = controls.astype(np.float64)
    ptr = ptr0.astype(np.float64).copy()
    P = np.empty((T, S), dtype=np.float64)
    U = np.empty((T, S), dtype=np.float64)
    for t in range(T):
        push, pop, noop = ctrl[t, 0], ctrl[t, 1], ctrl[t, 2]
        u = np.roll(ptr, 1)
        d = np.roll(ptr, -1)
        q = push * u + pop * d + noop * ptr
        ptr = q / (q.sum() + EPS)
        U[t] = u
        P[t] = ptr
    push_v = ctrl[:, 0]
    alpha = 1.0 - push_v
    W = np.zeros((T, T), dtype=np.float64)
    row = np.zeros(T, dtype=np.float64)
    for t in range(T):
        row *= alpha[t]
        row[t] = push_v[t]
        W[t] = row
    c = np.cumprod(alpha)
    return P, U, W, c


def _build_program():
    from contextlib import ExitStack

    import concourse.bass as bass
    import concourse.mybir as mybir

    f32 = mybir.dt.float32
    nc = bass.Bass("TRN2", num_devices=NCORES)

    pk = nc.declare_dram_parameter("pk", [P_PART, PK_W], f32, isOutput=False)
    out = nc.declare_dram_parameter("readst", [DSH, T], f32, isOutput=True)

    # Raw Bass with manual semaphores: the walrus build here only accepts
    # ONE sync wait per instruction, so every cross-engine dependency is a
    # standalone wait_ge followed by wait-free compute instructions.
    with ExitStack() as ctx:
        t_pk = ctx.enter_context(nc.sbuf_tensor("t_pk", [P_PART, PK_W], f32))
        t_ot = ctx.enter_context(nc.sbuf_tensor("t_ot", [DSH, T], f32))
        # one PSUM bank per 128-wide t-block so block copies can overlap
        # later matmuls (PE-write + DVE-read of the SAME bank is fatal)
        ps_b = [
            ctx.enter_context(nc.psum_tensor(f"ps_b{j}", [DSH, P_PART], f32))
            for j in range(N_KC)
        ]

        # DMA chunks, one sem each (SWDGE and HWDGE sems must not mix):
        #   gpsimd: cols 0:256 (V0|GT0 head) and 1152:1408 (V3|GT3)
        #   sync:   cols 256:768 (GT0 tail|V1|GT1)
        #   scalar: cols 768:1152 (V2|GT2)
        dA = ctx.enter_context(nc.semaphore("dA"))
        dB = ctx.enter_context(nc.semaphore("dB"))
        dC = ctx.enter_context(nc.semaphore("dC"))
        dD = ctx.enter_context(nc.semaphore("dD"))
        dOut = ctx.enter_context(nc.semaphore("dOut"))
        sPE = ctx.enter_context(nc.semaphore("sPE"))
        sDVE = ctx.enter_context(nc.semaphore("sDVE"))

        vs = [t_pk[:, _V_OFF[kc] : _V_OFF[kc] + DSH] for kc in range(N_KC)]
        gts = [t_pk[:, _G_OFF[kc] : _G_OFF[kc] + _G_W[kc]] for kc in range(N_KC)]

        def pk_chunk(eng, c0, c1, sem):
            eng.dma_start(t_pk[:, c0:c1], pk[:, c0:c1]).then_inc(sem, 16)

        # cumulative matmul count after each block completes
        cum = [1, 3, 5, 7]

        with nc.Block() as block:

            @block.gpsimd
            def _(g):
                pk_chunk(g, 0, 256, dA)
                pk_chunk(g, 1152, 1408, dD)
                for j in range(N_KC):
                    g.wait_ge(sDVE, j + 1)
                    g.dma_start(
                        out[:, j * P_PART : (j + 1) * P_PART],
                        t_ot[:, j * P_PART : (j + 1) * P_PART],
                    ).then_inc(dOut, 16)
                g.wait_ge(dOut, 16 * N_KC)

            @block.sync
            def _(sy):
                pk_chunk(sy, 256, 768, dB)

            @block.scalar
            def _(sc):
                pk_chunk(sc, 768, 1152, dC)

            @block.tensor
            def _(pe):
                # readsT block j = Σ_{kc in {j-1, j}} V[kc]^T GT[kc][:, block j]
                # — fp32, exact; one PSUM bank per block so each block
                # finalizes independently and copies overlap later matmuls.
                def mm(j, kc, start, stop):
                    c0 = j * P_PART - _G_T0[kc]
                    pe.matmul(
                        ps_b[j][:],
                        lhsT=vs[kc][:],
                        rhs=gts[kc][:, c0 : c0 + P_PART],
                        start=start,
                        stop=stop,
                    ).then_inc(sPE)

                pe.wait_ge(dA, 16)
                mm(0, 0, True, True)  # block 0 done (1)
                pe.wait_ge(dB, 16)
                mm(1, 0, True, False)
                mm(1, 1, False, True)  # block 1 done (3)
                mm(2, 1, True, False)
                pe.wait_ge(dC, 16)
                mm(2, 2, False, True)  # block 2 done (5)
                mm(3, 2, True, False)
                pe.wait_ge(dD, 16)
                mm(3, 3, False, True)  # block 3 done (7)

            @block.vector
            def _(v):
                for j in range(N_KC):
                    v.wait_ge(sPE, cum[j])
                    v.tensor_copy(
                        t_ot[:, j * P_PART : (j + 1) * P_PART], ps_b[j][:]
                    ).then_inc(sDVE)

    return nc


def _get_program():
    if "nc" not in _compiled:
        _compiled["nc"] = _build_program()
    return _compiled["nc"]


def _make_inputs(values, controls, mem0, ptr0):
    P, U, W, c = _host_precompute(controls, ptr0)
    G = (W * (P @ U.T)).astype(np.float32)  # [t, k], lower triangular
    GT = np.ascontiguousarray(G.T)  # [k, t]
    values = np.asarray(values, dtype=np.float32)

    def pack(core):
        vsh = values[:, core * DSH : (core + 1) * DSH]  # [T, DSH]
        parts = []
        for kc in range(N_KC):
            parts.append(vsh[kc * P_PART : (kc + 1) * P_PART, :])
            parts.append(
                GT[kc * P_PART : (kc + 1) * P_PART, _G_T0[kc] : _G_T0[kc] + _G_W[kc]]
            )
        return np.ascontiguousarray(np.concatenate(parts, axis=1), dtype=np.float32)

    in_maps = [{"pk": pack(i)} for i in range(NCORES)]
    # far-band residual (t - k >= 256 - _G_T0 offset): everything outside
    # the transferred blocks.  Decay makes it exactly zero in f32 for the
    # generator's distribution; guard anyway.
    Gfar = G.astype(np.float64).copy().T  # [k, t]
    for kc in range(N_KC):
        Gfar[kc * P_PART : (kc + 1) * P_PART, _G_T0[kc] : _G_T0[kc] + _G_W[kc]] = 0.0
    if np.any(Gfar):
        resid = (Gfar.T @ values.astype(np.float64)).astype(np.float32)
    else:
        resid = None
    return in_maps, P, c, resid


def kernel(values, controls, mem0, ptr0):
    from concourse.bass_utils import run_bass_kernel_spmd

    mem0 = np.asarray(mem0, dtype=np.float32)
    in_maps, P, c, resid = _make_inputs(values, controls, mem0, ptr0)
    nc = _get_program()
    res = run_bass_kernel_spmd(nc, in_maps, list(range(NCORES)))
    reads = np.concatenate(
        [np.asarray(res.results[i]["readst"]).T for i in range(NCORES)], axis=1
    )
    if resid is not None:
        reads = reads + resid
    if np.any(mem0):
        reads = reads + (c[:, None] * (P @ mem0.astype(np.float64))).astype(np.float32)
    ptrs = P.astype(np.float32)
    return reads, ptrs


# revision 28
# speedup vs baseline: 1.2763x; 1.1421x over previous
"""DifferentiableStack kernel for 8 Trainium2 NeuronCores.

Reformulation: unrolling the mem recurrence
    mem_t = push_t * outer(u_t, v_t) + (1 - push_t) * mem_{t-1}
    read_t = mem_t^T ptr_t
gives
    reads = G @ V + c ⊙ (P @ mem0),   G = W ⊙ (P U^T)  (lower triangular)
with
    P[t] = ptr_t            (normalized pointer after step t)
    U[t] = u_t = roll(ptr_{t-1}, 1)
    W[t,k] = push_k * Π_{j=k+1..t} (1 - push_j)   (k ≤ t, else 0)
    c[t]   = Π_{j=1..t} (1 - push_j)

The ptr chain / W / G are small sequential scalar recurrences plus one
[T,S]x[S,T] product, computed on the host in float64.  The heavy
O(T^2 D) contraction reads = G @ V runs on the TensorEngines in exact
fp32, sharded along the d (feature) axis: core i computes
readsT[d_shard, t] = Σ_k V[k, d_shard]^T G^T[k, t].  G^T is zero for
t < k and numerically zero for t - k >= 128 (exponential decay of the
blend weights), so only the diagonal and first sub-diagonal 128x128
blocks are transferred and multiplied.
"""

import sys

sys.path.insert(0, "/opt/trn_rl_repo")

import numpy as np

T, S, D = 512, 256, 1024
EPS = 1e-6
NCORES = 8
DSH = D // NCORES  # 128 features per core
P_PART = 128  # SBUF partitions
N_KC = T // P_PART  # 4 k-chunks

# G is numerically block-banded: W decays like Π(1-push_j), so entries
# with t - k >= 128 are < 1e-40 — only the diagonal and first
# sub-diagonal 128x128 blocks of G^T survive (verified < 1.6e-57
# residual on the generator's input distribution; any residual is
# added back on the host, see kernel()).
#
# packed input layout, one [128, 1408] f32 array per core:
#   | V0 | GT0 (t 0:256) | V1 | GT1 (t 128:384) | V2 | GT2 (t 256:512) | V3 | GT3 (t 384:512) |
# where Vk = V[k-chunk, d_shard] and GTk = G^T[k-chunk, t-band].
_V_OFF = [0, 384, 768, 1152]
_G_OFF = [128, 512, 896, 1280]
_G_T0 = [0, 128, 256, 384]
_G_W = [256, 256, 256, 128]
PK_W = 1408

_compiled = {}


def _host_precompute(controls, ptr0):
    """float64 ptr chain + blend-weight matrix. O(T*S + T^2) scalar work."""
    ctrl = controls.astype(np.float64)
    ptr = ptr0.astype(np.float64).copy()
    P = np.empty((T, S), dtype=np.float64)
    U = np.empty((T, S), dtype=np.float64)
    for t in range(T):
        push, pop, noop = ctrl[t, 0], ctrl[t, 1], ctrl[t, 2]
        u = np.roll(ptr, 1)
        d = np.roll(ptr, -1)
        q = push * u + pop * d + noop * ptr
        ptr = q / (q.sum() + EPS)
        U[t] = u
        P[t] = ptr
    push_v = ctrl[:, 0]
    alpha = 1.0 - push_v
    W = np.zeros((T, T), dtype=np.float64)
    row = np.zeros(T, dtype=np.float64)
    for t in range(T):
        row *= alpha[t]
        row[t] = push_v[t]
        W[t] = row
    c = np.cumprod(alpha)
    return P, U, W, c


def _build_program():
    from contextlib import ExitStack

    import concourse.bass as bass
    import concourse.mybir as mybir

    f32 = mybir.dt.float32
    nc = bass.Bass("TRN2", num_devices=NCORES)

    pk = nc.declare_dram_parameter("pk", [P_PART, PK_W], f32, isOutput=False)
    out = nc.declare_dram_parameter("readst", [DSH, T], f32, isOutput=True)

    # Raw Bass with manual semaphores: the walrus build here only accepts
    # ONE sync wait per instruction, so every cross-engine dependency is a
    # standalone wait_ge followed by wait-free compute instructions.
    with ExitStack() as ctx:
        t_pk = ctx.enter_context(nc.sbuf_tensor("t_pk", [P_PART, PK_W], f32))
        t_ot = ctx.enter_context(nc.sbuf_tensor("t_ot", [DSH, T], f32))
        # one PSUM bank per 128-wide t-block so block copies can overlap
        # later matmuls (PE-write + DVE-read of the SAME bank is fatal)
        ps_b = [
            ctx.enter_context(nc.psum_tensor(f"ps_b{j}", [DSH, P_PART], f32))
            for j in range(N_KC)
        ]
        ps_w = ctx.enter_context(nc.psum_tensor("ps_w", [DSH, 256], f32))

        # DMA chunks, one sem each (SWDGE and HWDGE sems must not mix):
        #   gpsimd: cols 0:256 (V0|GT0 head) and 1152:1408 (V3|GT3)
        #   sync:   cols 256:768 (GT0 tail|V1|GT1)
        #   scalar: cols 768:1152 (V2|GT2)
        dA = ctx.enter_context(nc.semaphore("dA"))
        dB = ctx.enter_context(nc.semaphore("dB"))
        dC = ctx.enter_context(nc.semaphore("dC"))
        dD = ctx.enter_context(nc.semaphore("dD"))
        dOutA = ctx.enter_context(nc.semaphore("dOutA"))
        dOutB = ctx.enter_context(nc.semaphore("dOutB"))
        sPE = ctx.enter_context(nc.semaphore("sPE"))
        sZ = ctx.enter_context(nc.semaphore("sZ"))
        sDVE = ctx.enter_context(nc.semaphore("sDVE"))

        vs = [t_pk[:, _V_OFF[kc] : _V_OFF[kc] + DSH] for kc in range(N_KC)]
        gts = [t_pk[:, _G_OFF[kc] : _G_OFF[kc] + _G_W[kc]] for kc in range(N_KC)]

        def pk_chunk(eng, c0, c1, sem):
            eng.dma_start(t_pk[:, c0:c1], pk[:, c0:c1]).then_inc(sem, 16)

        # cumulative matmul count after each block completes
        cum = [1, 3, 5, 7]

        with nc.Block() as block:

            def out_block(eng, j, sem):
                eng.wait_ge(sDVE, j + 1)
                eng.dma_start(
                    out[:, j * P_PART : (j + 1) * P_PART],
                    t_ot[:, j * P_PART : (j + 1) * P_PART],
                ).then_inc(sem, 16)

            @block.gpsimd
            def _(g):
                pk_chunk(g, 0, 256, dA)
                pk_chunk(g, 1152, 1408, dD)

            @block.sync
            def _(sy):
                pk_chunk(sy, 256, 768, dB)
                out_block(sy, 0, dOutA)
                out_block(sy, 2, dOutA)
                sy.wait_ge(dOutA, 32)

            @block.scalar
            def _(sc):
                pk_chunk(sc, 768, 1152, dC)
                out_block(sc, 1, dOutB)
                out_block(sc, 3, dOutB)
                sc.wait_ge(dOutB, 32)

            @block.tensor
            def _(pe):
                # readsT block j = Σ_{kc in {j-1, j}} V[kc]^T GT[kc][:, block j]
                # — fp32, exact; one PSUM bank per block so each block
                # finalizes independently and copies overlap later matmuls.
                def mm(j, kc, start, stop):
                    c0 = j * P_PART - _G_T0[kc]
                    pe.matmul(
                        ps_b[j][:],
                        lhsT=vs[kc][:],
                        rhs=gts[kc][:, c0 : c0 + P_PART],
                        start=start,
                        stop=stop,
                    ).then_inc(sPE)

                # warm-up: dummy matmuls into a scratch bank while the
                # input DMAs are in flight — gets the PE past its p-state
                # / HAM ramp so the real matmuls run at full rate.  t_ot
                # (memset by the DVE first) is not written again until
                # after the real matmuls start, so reading it is race-free
                # (values are irrelevant).
                pe.wait_ge(sZ, 1)
                for _ in range(4):
                    pe.matmul(
                        ps_w[:],
                        lhsT=t_ot[:, 0:P_PART],
                        rhs=t_ot[:, 0:256],
                        start=True,
                        stop=True,
                        skip_group_check=True,
                    )
                pe.wait_ge(dA, 16)
                mm(0, 0, True, True)  # block 0 done (1)
                pe.wait_ge(dB, 16)
                mm(1, 0, True, False)
                mm(1, 1, False, True)  # block 1 done (3)
                mm(2, 1, True, False)
                pe.wait_ge(dC, 16)
                mm(2, 2, False, True)  # block 2 done (5)
                mm(3, 2, True, False)
                pe.wait_ge(dD, 16)
                mm(3, 3, False, True)  # block 3 done (7)

            @block.vector
            def _(v):
                v.memset(t_ot[:, 0:256], 0.0).then_inc(sZ)
                for j in range(N_KC):
                    v.wait_ge(sPE, cum[j])
                    v.tensor_copy(
                        t_ot[:, j * P_PART : (j + 1) * P_PART], ps_b[j][:]
                    ).then_inc(sDVE)

    return nc


def _get_program():
    if "nc" not in _compiled:
        _compiled["nc"] = _build_program()
    return _compiled["nc"]


def _make_inputs(values, controls, mem0, ptr0):
    P, U, W, c = _host_precompute(controls, ptr0)
    G = (W * (P @ U.T)).astype(np.float32)  # [t, k], lower triangular
    GT = np.ascontiguousarray(G.T)  # [k, t]
    values = np.asarray(values, dtype=np.float32)

    def pack(core):
        vsh = values[:, core * DSH : (core + 1) * DSH]  # [T, DSH]
        parts = []
        for kc in range(N_KC):
            parts.append(vsh[kc * P_PART : (kc + 1) * P_PART, :])
            parts.append(
                GT[kc * P_PART : (kc + 1) * P_PART, _G_T0[kc] : _G_T0[kc] + _G_W[kc]]
            )
        return np.ascontiguousarray(np.concatenate(parts, axis=1), dtype=np.float32)

    in_maps = [{"pk": pack(i)} for i in range(NCORES)]
    # far-band residual (t - k >= 256 - _G_T0 offset): everything outside
    # the transferred blocks.  Decay makes it exactly zero in f32 for the
    # generator's distribution; guard anyway.
    Gfar = G.astype(np.float64).copy().T  # [k, t]
    for kc in range(N_KC):
        Gfar[kc * P_PART : (kc + 1) * P_PART, _G_T0[kc] : _G_T0[kc] + _G_W[kc]] = 0.0
    if np.any(Gfar):
        resid = (Gfar.T @ values.astype(np.float64)).astype(np.float32)
    else:
        resid = None
    return in_maps, P, c, resid


def kernel(values, controls, mem0, ptr0):
    from concourse.bass_utils import run_bass_kernel_spmd

    mem0 = np.asarray(mem0, dtype=np.float32)
    in_maps, P, c, resid = _make_inputs(values, controls, mem0, ptr0)
    nc = _get_program()
    res = run_bass_kernel_spmd(nc, in_maps, list(range(NCORES)))
    reads = np.concatenate(
        [np.asarray(res.results[i]["readst"]).T for i in range(NCORES)], axis=1
    )
    if resid is not None:
        reads = reads + resid
    if np.any(mem0):
        reads = reads + (c[:, None] * (P @ mem0.astype(np.float64))).astype(np.float32)
    ptrs = P.astype(np.float32)
    return reads, ptrs


# revision 29
# speedup vs baseline: 1.4173x; 1.1105x over previous
"""DifferentiableStack kernel for 8 Trainium2 NeuronCores.

Reformulation: unrolling the mem recurrence
    mem_t = push_t * outer(u_t, v_t) + (1 - push_t) * mem_{t-1}
    read_t = mem_t^T ptr_t
gives
    reads = G @ V + c ⊙ (P @ mem0),   G = W ⊙ (P U^T)  (lower triangular)
with
    P[t] = ptr_t            (normalized pointer after step t)
    U[t] = u_t = roll(ptr_{t-1}, 1)
    W[t,k] = push_k * Π_{j=k+1..t} (1 - push_j)   (k ≤ t, else 0)
    c[t]   = Π_{j=1..t} (1 - push_j)

The ptr chain / W / G are small sequential scalar recurrences plus one
[T,S]x[S,T] product, computed on the host in float64.  The heavy
O(T^2 D) contraction reads = G @ V runs on the TensorEngines in exact
fp32, sharded along the d (feature) axis: core i computes
readsT[d_shard, t] = Σ_k V[k, d_shard]^T G^T[k, t].  G^T is zero for
t < k and numerically zero for t - k >= 128 (exponential decay of the
blend weights), so only the diagonal and first sub-diagonal 128x128
blocks are transferred and multiplied.
"""

import sys

sys.path.insert(0, "/opt/trn_rl_repo")

import numpy as np

T, S, D = 512, 256, 1024
EPS = 1e-6
NCORES = 8
DSH = D // NCORES  # 128 features per core
P_PART = 128  # SBUF partitions
N_KC = T // P_PART  # 4 k-chunks

# G is numerically block-banded: W decays like Π(1-push_j), so entries
# with t - k >= 128 are < 1e-40 — only the diagonal and first
# sub-diagonal 128x128 blocks of G^T survive (verified < 1.6e-57
# residual on the generator's input distribution; any residual is
# added back on the host, see kernel()).
#
# packed input layout, one [128, 1408] f32 array per core:
#   | V0 | GT0 (t 0:256) | V1 | GT1 (t 128:384) | V2 | GT2 (t 256:512) | V3 | GT3 (t 384:512) |
# where Vk = V[k-chunk, d_shard] and GTk = G^T[k-chunk, t-band].
_V_OFF = [0, 384, 768, 1152]
_G_OFF = [128, 512, 896, 1280]
_G_T0 = [0, 128, 256, 384]
_G_W = [256, 256, 256, 128]
PK_W = 1408

_compiled = {}


def _host_precompute(controls, ptr0):
    """float64 ptr chain + blend-weight matrix. O(T*S + T^2) scalar work."""
    ctrl = controls.astype(np.float64)
    ptr = ptr0.astype(np.float64).copy()
    P = np.empty((T, S), dtype=np.float64)
    U = np.empty((T, S), dtype=np.float64)
    for t in range(T):
        push, pop, noop = ctrl[t, 0], ctrl[t, 1], ctrl[t, 2]
        u = np.roll(ptr, 1)
        d = np.roll(ptr, -1)
        q = push * u + pop * d + noop * ptr
        ptr = q / (q.sum() + EPS)
        U[t] = u
        P[t] = ptr
    push_v = ctrl[:, 0]
    alpha = 1.0 - push_v
    W = np.zeros((T, T), dtype=np.float64)
    row = np.zeros(T, dtype=np.float64)
    for t in range(T):
        row *= alpha[t]
        row[t] = push_v[t]
        W[t] = row
    c = np.cumprod(alpha)
    return P, U, W, c


def _build_program():
    from contextlib import ExitStack

    import concourse.bass as bass
    import concourse.mybir as mybir

    f32 = mybir.dt.float32
    nc = bass.Bass("TRN2", num_devices=NCORES)

    pk = nc.declare_dram_parameter("pk", [P_PART, PK_W], f32, isOutput=False)
    out = nc.declare_dram_parameter("readst", [DSH, T], f32, isOutput=True)

    # Raw Bass with manual semaphores: the walrus build here only accepts
    # ONE sync wait per instruction, so every cross-engine dependency is a
    # standalone wait_ge followed by wait-free compute instructions.
    with ExitStack() as ctx:
        t_pk = ctx.enter_context(nc.sbuf_tensor("t_pk", [P_PART, PK_W], f32))
        t_ot = ctx.enter_context(nc.sbuf_tensor("t_ot", [DSH, T], f32))
        # one PSUM bank per 128-wide t-block so block copies can overlap
        # later matmuls (PE-write + DVE-read of the SAME bank is fatal)
        ps_b = [
            ctx.enter_context(nc.psum_tensor(f"ps_b{j}", [DSH, P_PART], f32))
            for j in range(N_KC)
        ]
        ps_w = ctx.enter_context(nc.psum_tensor("ps_w", [DSH, 256], f32))

        # DMA chunks, one sem each (SWDGE and HWDGE sems must not mix):
        #   gpsimd: cols 0:256 (V0|GT0 head) and 1152:1408 (V3|GT3)
        #   sync:   cols 256:768 (GT0 tail|V1|GT1)
        #   scalar: cols 768:1152 (V2|GT2)
        dA = ctx.enter_context(nc.semaphore("dA"))
        dB = ctx.enter_context(nc.semaphore("dB"))
        dC = ctx.enter_context(nc.semaphore("dC"))
        dD = ctx.enter_context(nc.semaphore("dD"))
        dOutA = ctx.enter_context(nc.semaphore("dOutA"))
        dOutB = ctx.enter_context(nc.semaphore("dOutB"))
        sPE = ctx.enter_context(nc.semaphore("sPE"))
        sZ = ctx.enter_context(nc.semaphore("sZ"))
        sDVE = ctx.enter_context(nc.semaphore("sDVE"))

        vs = [t_pk[:, _V_OFF[kc] : _V_OFF[kc] + DSH] for kc in range(N_KC)]
        gts = [t_pk[:, _G_OFF[kc] : _G_OFF[kc] + _G_W[kc]] for kc in range(N_KC)]

        def pk_chunk(eng, c0, c1, sem):
            eng.dma_start(t_pk[:, c0:c1], pk[:, c0:c1]).then_inc(sem, 16)

        # cumulative matmul count after each block completes
        cum = [1, 3, 5, 7]

        with nc.Block() as block:

            @block.gpsimd
            def _(g):
                pk_chunk(g, 0, 256, dA)
                pk_chunk(g, 1152, 1408, dD)

            @block.sync
            def _(sy):
                pk_chunk(sy, 256, 768, dB)
                sy.wait_ge(sDVE, 2)
                sy.dma_start(out[:, 0:256], t_ot[:, 0:256]).then_inc(dOutA, 16)
                sy.wait_ge(dOutA, 16)

            @block.scalar
            def _(sc):
                pk_chunk(sc, 768, 1152, dC)
                sc.wait_ge(sDVE, 4)
                sc.dma_start(out[:, 256:T], t_ot[:, 256:T]).then_inc(dOutB, 16)
                sc.wait_ge(dOutB, 16)

            @block.tensor
            def _(pe):
                # readsT block j = Σ_{kc in {j-1, j}} V[kc]^T GT[kc][:, block j]
                # — fp32, exact; one PSUM bank per block so each block
                # finalizes independently and copies overlap later matmuls.
                def mm(j, kc, start, stop):
                    c0 = j * P_PART - _G_T0[kc]
                    pe.matmul(
                        ps_b[j][:],
                        lhsT=vs[kc][:],
                        rhs=gts[kc][:, c0 : c0 + P_PART],
                        start=start,
                        stop=stop,
                    ).then_inc(sPE)

                # warm-up: dummy matmuls into a scratch bank while the
                # input DMAs are in flight — gets the PE past its p-state
                # / HAM ramp so the real matmuls run at full rate.  t_ot
                # (memset by the DVE first) is not written again until
                # after the real matmuls start, so reading it is race-free
                # (values are irrelevant).
                pe.wait_ge(sZ, 1)
                for _ in range(3):
                    pe.matmul(
                        ps_w[:],
                        lhsT=t_ot[:, 0:P_PART],
                        rhs=t_ot[:, 0:256],
                        start=True,
                        stop=True,
                        skip_group_check=True,
                    )
                pe.wait_ge(dA, 16)
                mm(0, 0, True, True)  # block 0 done (1)
                pe.wait_ge(dB, 16)
                mm(1, 0, True, False)
                mm(1, 1, False, True)  # block 1 done (3)
                mm(2, 1, True, False)
                pe.wait_ge(dC, 16)
                mm(2, 2, False, True)  # block 2 done (5)
                mm(3, 2, True, False)
                pe.wait_ge(dD, 16)
                mm(3, 3, False, True)  # block 3 done (7)

            @block.vector
            def _(v):
                v.memset(t_ot[:, 0:256], 0.0).then_inc(sZ)
                for j in range(N_KC):
                    v.wait_ge(sPE, cum[j])
                    v.tensor_copy(
                        t_ot[:, j * P_PART : (j + 1) * P_PART], ps_b[j][:]
                    ).then_inc(sDVE)

    return nc


def _get_program():
    if "nc" not in _compiled:
        _compiled["nc"] = _build_program()
    return _compiled["nc"]


def _make_inputs(values, controls, mem0, ptr0):
    P, U, W, c = _host_precompute(controls, ptr0)
    G = (W * (P @ U.T)).astype(np.float32)  # [t, k], lower triangular
    GT = np.ascontiguousarray(G.T)  # [k, t]
    values = np.asarray(values, dtype=np.float32)

    def pack(core):
        vsh = values[:, core * DSH : (core + 1) * DSH]  # [T, DSH]
        parts = []
        for kc in range(N_KC):
            parts.append(vsh[kc * P_PART : (kc + 1) * P_PART, :])
            parts.append(
                GT[kc * P_PART : (kc + 1) * P_PART, _G_T0[kc] : _G_T0[kc] + _G_W[kc]]
            )
        return np.ascontiguousarray(np.concatenate(parts, axis=1), dtype=np.float32)

    in_maps = [{"pk": pack(i)} for i in range(NCORES)]
    # far-band residual (t - k >= 256 - _G_T0 offset): everything outside
    # the transferred blocks.  Decay makes it exactly zero in f32 for the
    # generator's distribution; guard anyway.
    Gfar = G.astype(np.float64).copy().T  # [k, t]
    for kc in range(N_KC):
        Gfar[kc * P_PART : (kc + 1) * P_PART, _G_T0[kc] : _G_T0[kc] + _G_W[kc]] = 0.0
    if np.any(Gfar):
        resid = (Gfar.T @ values.astype(np.float64)).astype(np.float32)
    else:
        resid = None
    return in_maps, P, c, resid


def kernel(values, controls, mem0, ptr0):
    from concourse.bass_utils import run_bass_kernel_spmd

    mem0 = np.asarray(mem0, dtype=np.float32)
    in_maps, P, c, resid = _make_inputs(values, controls, mem0, ptr0)
    nc = _get_program()
    res = run_bass_kernel_spmd(nc, in_maps, list(range(NCORES)))
    reads = np.concatenate(
        [np.asarray(res.results[i]["readst"]).T for i in range(NCORES)], axis=1
    )
    if resid is not None:
        reads = reads + resid
    if np.any(mem0):
        reads = reads + (c[:, None] * (P @ mem0.astype(np.float64))).astype(np.float32)
    ptrs = P.astype(np.float32)
    return reads, ptrs
